# revision 12
# baseline (speedup 1.0000x reference)
"""Trainium2 Bass kernel for a pairwise-distance cluster margin loss.

Math (matches the jax reference):
    far_i  = max_{j: t_j=t_i} dist_ij
    near_i = second smallest dist_ij over class(i)  (smallest is self)
    loss   = mean(relu(far - near))

Key insight: the loss only involves SAME-CLASS distances.  With rows
sorted by class, each 128-row tile's class-mates lie within a narrow
band of the sorted order (max class size ~82), so each tile only needs
W ~ 264 columns instead of 4096 -> ~14x less GEMM work than the full
distance matrix.

Per core (512 sorted rows): the column "universe" is the sorted slice
order[512c-SPL : 512c-SPL+NCOL] (padded with zeros at the array ends).
Row-tile mt multiplies against universe cols [128mt, 128mt+W).  A single
fp8 tensor xt8 = fp8(sqrt2*x[universe])^T serves as BOTH matmul operands
(lhsT slice = own rows, rhs slice = window), so the PE computes
    psA = 2 x_i.x_j - sq_j - C*mask      (fp8 DR chain + one bf16 aug)
and the stats flip max<->min versus the usual formulation:
    rowmin(psA)                   -> far2  = sq_i - C - fstat
    rowmax(psA + 2C*mask + Ddiag) -> near2 = sq_i + C - gstat
The mask/diag term is a host-precomputed bf16 SBUF tensor added on the
DVE (no second matmul chain, no scalar-engine copy).  Host applies
sqrt / relu / mean on the 4096 reduced stats.
"""

import numpy as np
import ml_dtypes

BF = ml_dtypes.bfloat16
F8 = ml_dtypes.float8_e4m3

N = 4096  # rows (points)
D = 2048  # feature dim
P = 128  # partitions
NCORES = 8
MB = N // NCORES  # 512 rows per core
KX = D // P  # 16 x-chunks of 128
MT = MB // P  # 4 row tiles of 128 per core
NCLS = 64

C = float(2.0**17)  # mask offset; > max |2xixj - sqj| (~15k)
DIAG = -float(2.0**31)  # diagonal push-out

_compiled = None  # (key, nc)


def _spill(ts):
    """Max class-band spill (left, right) over all 128-row windows of the
    class-sorted target vector ts."""
    spl = spr = 0
    nw = N // P
    for w in range(nw):
        lo_cls = ts[w * P]
        hi_cls = ts[w * P + P - 1]
        lo = int(np.searchsorted(ts, lo_cls, "left"))
        hi = int(np.searchsorted(ts, hi_cls, "right"))
        spl = max(spl, w * P - lo)
        spr = max(spr, hi - (w * P + P))
    return spl, spr


def _build_nc(SPL, W, NCOL):
    import concourse.mybir as mybir
    import concourse.tile as tile
    from concourse import bacc

    nc = bacc.Bacc("TRN2", target_bir_lowering=False)
    f32 = mybir.dt.float32
    bf16 = mybir.dt.bfloat16
    fp8 = mybir.dt.float8e4
    DR = mybir.MatmulPerfMode.DoubleRow
    X = mybir.AxisListType.X
    MIN = mybir.AluOpType.min

    # packed bf16 tensor: [lhsa (MB) | aug (NCOL) | psbp (MT*W)]
    LHSA = 0
    AUG = MB
    PSBP = MB + NCOL
    PK = MB + NCOL + MT * W

    xt_d = nc.dram_tensor("xt", [P, KX, NCOL], fp8, kind="ExternalInput")
    pk_d = nc.dram_tensor("pk", [P, PK], bf16, kind="ExternalInput")
    res_d = nc.dram_tensor("res", [P, 2 * MT], f32, kind="ExternalOutput")

    with tile.TileContext(nc) as tc:
        with (
            tc.tile_pool(name="singles", bufs=1) as singles,
            tc.tile_pool(name="psa", bufs=1, space="PSUM") as psa,
            tc.tile_pool(name="gsc", bufs=2) as gsc,
        ):
            xt = singles.tile([P, KX, NCOL], fp8)
            pk = singles.tile([P, PK], bf16)
            # just-in-time DMA across three engine queues: chunk pairs land
            # roughly in consumption order of the chunk-major matmul loop
            nc.sync.dma_start(out=xt[:, 0:2, :], in_=xt_d[:, 0:2, :])
            nc.scalar.dma_start(out=pk, in_=pk_d[:, :])
            nc.gpsimd.dma_start(out=xt[:, 2:4, :], in_=xt_d[:, 2:4, :])
            nc.sync.dma_start(out=xt[:, 4:8, :], in_=xt_d[:, 4:8, :])
            nc.scalar.dma_start(out=xt[:, 8:12, :], in_=xt_d[:, 8:12, :])
            nc.sync.dma_start(out=xt[:, 12:KX, :], in_=xt_d[:, 12:KX, :])

            fg = singles.tile([P, 2 * MT], f32, name="fg")

            at = [psa.tile([P, 512], f32, name=f"a{mt}") for mt in range(MT)]
            # aug matmuls first: they only need pk, which lands before xt
            for mt in range(MT):
                nc.tensor.matmul(
                    at[mt][:, 0:W],
                    pk[:, LHSA + mt * P : LHSA + (mt + 1) * P],
                    pk[:, AUG + mt * P : AUG + mt * P + W],
                    start=True,
                    stop=False,
                )
            # chunk-major fp8 DR chains into the four PSUM banks
            for c in range(0, KX, 2):
                for mt in range(MT):
                    nc.tensor.matmul(
                        at[mt][:, 0:W],
                        xt[:, c : c + 2, SPL + mt * P : SPL + mt * P + P],
                        xt[:, c : c + 2, mt * P : mt * P + W],
                        start=False,
                        stop=(c == KX - 2),
                        perf_mode=DR,
                    )
            for mt in range(MT):
                a = at[mt]
                g = gsc.tile([P, W], f32)
                nc.vector.tensor_add(
                    g, pk[:, PSBP + mt * W : PSBP + (mt + 1) * W], a[:, 0:W]
                )
                nc.vector.reduce_max(fg[:, MT + mt : MT + mt + 1], g, axis=X)
                nc.vector.tensor_reduce(
                    fg[:, mt : mt + 1], a[:, 0:W], axis=X, op=MIN
                )

            nc.sync.dma_start(out=res_d[:, :], in_=fg)

    nc.compile()
    return nc


def _prep(x, t):
    x = np.asarray(x, np.float32)
    t = np.asarray(t).astype(np.int64)
    order = np.argsort(t, kind="stable")
    ts = t[order]
    spl, spr = _spill(ts)
    W = ((P + spl + spr) + 7) // 8 * 8
    NCOL = (MB + (W - P) + 63) // 64 * 64  # 64-col aligned for LDWEIGHTS
    SPL = spl

    q8 = (np.float32(np.sqrt(2.0)) * x).astype(F8)  # [N, D]
    sq = np.sum(x.astype(np.float64) ** 2, axis=1)
    sqhi = sq.astype(BF)
    sqlo = (sq - sqhi.astype(np.float64)).astype(BF)

    LHSA = 0
    AUG = MB
    PSBP = MB + NCOL
    PK = MB + NCOL + MT * W

    in_maps = []
    meta = []
    for c0 in range(NCORES):
        u0 = c0 * MB - SPL
        uidx = np.arange(u0, u0 + NCOL)
        valid = (uidx >= 0) & (uidx < N)
        gu = order[np.clip(uidx, 0, N - 1)]
        tu = np.where(valid, ts[np.clip(uidx, 0, N - 1)], -1)

        xt_cols = q8[gu].T.copy()  # [D, NCOL]
        xt_cols[:, ~valid] = F8(0.0)
        xt_np = np.ascontiguousarray(
            xt_cols.reshape(KX, P, NCOL).transpose(1, 0, 2)
        )

        pk_np = np.zeros((P, PK), BF)
        # lhsa block: row0 = row1 = 1, rows 2+c = onehot(t_row)
        rows = order[c0 * MB : (c0 + 1) * MB]
        ohr = np.zeros((NCLS, MB), np.float32)
        ohr[t[rows], np.arange(MB)] = 1.0
        pk_np[0, LHSA : LHSA + MB] = BF(1.0)
        pk_np[1, LHSA : LHSA + MB] = BF(1.0)
        pk_np[2 : 2 + NCLS, LHSA : LHSA + MB] = ohr.astype(BF)
        # aug block: row0 = -sqhi_j, row1 = -sqlo_j, rows 2+c = -C*onehot
        pk_np[0, AUG : AUG + NCOL] = np.where(valid, -sqhi[gu], BF(0.0))
        pk_np[1, AUG : AUG + NCOL] = np.where(valid, -sqlo[gu], BF(0.0))
        oh = np.zeros((NCLS, NCOL), np.float32)
        oh[tu[valid], np.nonzero(valid)[0]] = 1.0
        pk_np[2 : 2 + NCLS, AUG : AUG + NCOL] = (-C * oh).astype(BF)
        # psbp blocks: per tile, 2C*mask with diag overwritten to DIAG
        for mt in range(MT):
            tr = tu[SPL + mt * P : SPL + mt * P + P]  # row classes
            tc_ = tu[mt * P : mt * P + W]  # window col classes
            m = (tr[:, None] == tc_[None, :]) & (tr[:, None] >= 0)
            blk = np.where(m, np.float32(2.0 * C), np.float32(0.0))
            blk[np.arange(P), SPL + np.arange(P)] = np.float32(DIAG)
            pk_np[:, PSBP + mt * W : PSBP + (mt + 1) * W] = blk.astype(BF)

        in_maps.append({"xt": xt_np, "pk": pk_np})
        meta.append(rows)
    return in_maps, meta, sq, (SPL, W, NCOL)


def _assemble(results, meta, sq):
    far2 = np.empty(N, np.float64)
    near2 = np.empty(N, np.float64)
    for c0 in range(NCORES):
        r = np.asarray(results[c0]["res"], np.float64)  # [P, 2*MT]
        rows = meta[c0]
        for mt in range(MT):
            g = rows[mt * P : (mt + 1) * P]
            far2[g] = sq[g] - C - r[:, mt]
            near2[g] = sq[g] + C - r[:, MT + mt]
    far = np.sqrt(np.maximum(far2, 0.0))
    near = np.sqrt(np.maximum(near2, 0.0))
    loss = np.float32(np.mean(np.maximum(far - near, 0.0)))
    return np.asarray(loss, np.float32)


def run_kernel(inputs, targets, trace=False):
    """Returns (loss, BassKernelResults)."""
    from concourse.bass_utils import run_bass_kernel_spmd

    global _compiled
    in_maps, meta, sq, key = _prep(inputs, targets)
    if _compiled is None or _compiled[0] != key:
        _compiled = (key, _build_nc(*key))
    nc = _compiled[1]
    br = run_bass_kernel_spmd(
        nc, in_maps, core_ids=list(range(NCORES)), trace=trace
    )
    return _assemble(br.results, meta, sq), br


def kernel(inputs, targets):
    loss, _ = run_kernel(inputs, targets)
    return loss


# revision 15
# speedup vs baseline: 1.0183x; 1.0183x over previous
"""Trainium2 Bass kernel for a pairwise-distance cluster margin loss.

Math (matches the jax reference):
    far_i  = max_{j: t_j=t_i} dist_ij
    near_i = second smallest dist_ij over class(i)  (smallest is self)
    loss   = mean(relu(far - near))

Key insight: the loss only involves SAME-CLASS distances.  With rows
sorted by class, each 128-row tile's class-mates lie within a narrow
band of the sorted order (max class size ~82), so each tile only needs
W ~ 264 columns instead of 4096 -> ~14x less GEMM work than the full
distance matrix.

Per core (512 sorted rows): the column "universe" is the sorted slice
order[512c-SPL : 512c-SPL+NCOL] (padded with zeros at the array ends).
Row-tile mt multiplies against universe cols [128mt, 128mt+W).  A single
fp8 tensor xt8 = fp8(sqrt2*x[universe])^T serves as BOTH matmul operands
(lhsT slice = own rows, rhs slice = window), so the PE computes
    psA = 2 x_i.x_j - sq_j - C*mask      (fp8 DR chain + one bf16 aug)
and the stats flip max<->min versus the usual formulation:
    rowmin(psA)                   -> far2  = sq_i - C - fstat
    rowmax(psA + 2C*mask + Ddiag) -> near2 = sq_i + C - gstat
The mask/diag term is a host-precomputed bf16 SBUF tensor added on the
DVE (no second matmul chain, no scalar-engine copy).  Host applies
sqrt / relu / mean on the 4096 reduced stats.
"""

import numpy as np
import ml_dtypes

BF = ml_dtypes.bfloat16
F8 = ml_dtypes.float8_e4m3

N = 4096  # rows (points)
D = 2048  # feature dim
P = 128  # partitions
NCORES = 8
MB = N // NCORES  # 512 rows per core
KX = D // P  # 16 x-chunks of 128
MT = MB // P  # 4 row tiles of 128 per core
NCLS = 64

C = float(2.0**17)  # mask offset; > max |2xixj - sqj| (~15k)
DIAG = -float(2.0**31)  # diagonal push-out

_compiled = None  # (key, nc)


def _spill(ts):
    """Max class-band spill (left, right) over all 128-row windows of the
    class-sorted target vector ts."""
    spl = spr = 0
    nw = N // P
    for w in range(nw):
        lo_cls = ts[w * P]
        hi_cls = ts[w * P + P - 1]
        lo = int(np.searchsorted(ts, lo_cls, "left"))
        hi = int(np.searchsorted(ts, hi_cls, "right"))
        spl = max(spl, w * P - lo)
        spr = max(spr, hi - (w * P + P))
    return spl, spr


def _build_nc(SPL, W, NCOL):
    import concourse.mybir as mybir
    import concourse.tile as tile
    from concourse import bacc

    nc = bacc.Bacc("TRN2", target_bir_lowering=False)
    f32 = mybir.dt.float32
    bf16 = mybir.dt.bfloat16
    fp8 = mybir.dt.float8e4
    DR = mybir.MatmulPerfMode.DoubleRow
    X = mybir.AxisListType.X
    MIN = mybir.AluOpType.min

    # packed bf16 tensor: [lhsa (MB) | aug (NCOL) | psbp (MT*W) | eye (P)]
    LHSA = 0
    AUG = MB
    PSBP = MB + NCOL
    EYE = MB + NCOL + MT * W
    PK = MB + NCOL + MT * W + P

    xt_d = nc.dram_tensor("xt", [P, KX, NCOL], fp8, kind="ExternalInput")
    pk_d = nc.dram_tensor("pk", [P, PK], bf16, kind="ExternalInput")
    res_d = nc.dram_tensor("res", [P, 2 * MT], f32, kind="ExternalOutput")

    with tile.TileContext(nc) as tc:
        with (
            tc.tile_pool(name="singles", bufs=1) as singles,
            tc.tile_pool(name="psa", bufs=1, space="PSUM") as psa,
        ):
            xt = singles.tile([P, KX, NCOL], fp8)
            pk = singles.tile([P, PK], bf16)
            # just-in-time DMA, balanced across the three DMA-capable
            # engine queues (~135 GB/s each): chunk pairs land roughly in
            # the consumption order of the tile-major matmul chains
            nc.sync.dma_start(out=xt[:, 0:2, :], in_=xt_d[:, 0:2, :])
            nc.gpsimd.dma_start(out=xt[:, 2:6, :], in_=xt_d[:, 2:6, :])
            nc.scalar.dma_start(out=pk, in_=pk_d[:, :])
            nc.sync.dma_start(out=xt[:, 6:10, :], in_=xt_d[:, 6:10, :])
            nc.gpsimd.dma_start(out=xt[:, 10:14, :], in_=xt_d[:, 10:14, :])
            nc.sync.dma_start(out=xt[:, 14:KX, :], in_=xt_d[:, 14:KX, :])

            fg = singles.tile([P, 2 * MT], f32, name="fg")

            at = [psa.tile([P, 512], f32, name=f"a{mt}") for mt in range(MT)]

            def chain(mt):
                m0 = SPL + mt * P
                c0 = mt * P
                a = at[mt]
                for c in range(0, KX, 2):
                    nc.tensor.matmul(
                        a[:, 0:W],
                        xt[:, c : c + 2, m0 : m0 + P],
                        xt[:, c : c + 2, c0 : c0 + W],
                        start=(c == 0),
                        stop=False,
                        perf_mode=DR,
                    )
                nc.tensor.matmul(
                    a[:, 0:W],
                    pk[:, LHSA + mt * P : LHSA + (mt + 1) * P],
                    pk[:, AUG + c0 : AUG + c0 + W],
                    start=False,
                    stop=True,
                )
                # far stat: rowmin of psA (diag never wins the min)
                nc.vector.tensor_reduce(
                    fg[:, mt : mt + 1], at[mt][:, 0:W], axis=X, op=MIN
                )

            def eyeadd(mt):
                # after the f-reduce has read psA, accumulate the
                # 2C*mask + DIAG*diag term into the same PSUM bank on the
                # PE (WAR dep is tracked by the tile framework) ...
                nc.tensor.matmul(
                    at[mt][:, 0:W],
                    pk[:, EYE : EYE + P],
                    pk[:, PSBP + mt * W : PSBP + (mt + 1) * W],
                    start=False,
                    stop=True,
                )
                # ... then the near stat is a plain rowmax of psA
                nc.vector.reduce_max(
                    fg[:, MT + mt : MT + mt + 1], at[mt][:, 0:W], axis=X
                )

            chain(0)
            chain(1)
            eyeadd(0)
            chain(2)
            eyeadd(1)
            chain(3)
            eyeadd(2)
            eyeadd(3)

            nc.sync.dma_start(out=res_d[:, :], in_=fg)

    nc.compile()
    return nc


def _prep(x, t):
    x = np.asarray(x, np.float32)
    t = np.asarray(t).astype(np.int64)
    order = np.argsort(t, kind="stable")
    ts = t[order]
    spl, spr = _spill(ts)
    W = ((P + spl + spr) + 7) // 8 * 8
    NCOL = (MB + (W - P) + 63) // 64 * 64  # 64-col aligned for LDWEIGHTS
    SPL = spl

    q8 = (np.float32(np.sqrt(2.0)) * x).astype(F8)  # [N, D]
    sq = np.sum(x.astype(np.float64) ** 2, axis=1)
    sqhi = sq.astype(BF)
    sqlo = (sq - sqhi.astype(np.float64)).astype(BF)

    LHSA = 0
    AUG = MB
    PSBP = MB + NCOL
    EYE = MB + NCOL + MT * W
    PK = MB + NCOL + MT * W + P

    in_maps = []
    meta = []
    for c0 in range(NCORES):
        u0 = c0 * MB - SPL
        uidx = np.arange(u0, u0 + NCOL)
        valid = (uidx >= 0) & (uidx < N)
        gu = order[np.clip(uidx, 0, N - 1)]
        tu = np.where(valid, ts[np.clip(uidx, 0, N - 1)], -1)

        xt_cols = q8[gu].T.copy()  # [D, NCOL]
        xt_cols[:, ~valid] = F8(0.0)
        xt_np = np.ascontiguousarray(
            xt_cols.reshape(KX, P, NCOL).transpose(1, 0, 2)
        )

        pk_np = np.zeros((P, PK), BF)
        # lhsa block: row0 = row1 = 1, rows 2+c = onehot(t_row)
        rows = order[c0 * MB : (c0 + 1) * MB]
        ohr = np.zeros((NCLS, MB), np.float32)
        ohr[t[rows], np.arange(MB)] = 1.0
        pk_np[0, LHSA : LHSA + MB] = BF(1.0)
        pk_np[1, LHSA : LHSA + MB] = BF(1.0)
        pk_np[2 : 2 + NCLS, LHSA : LHSA + MB] = ohr.astype(BF)
        # aug block: row0 = -sqhi_j, row1 = -sqlo_j, rows 2+c = -C*onehot
        pk_np[0, AUG : AUG + NCOL] = np.where(valid, -sqhi[gu], BF(0.0))
        pk_np[1, AUG : AUG + NCOL] = np.where(valid, -sqlo[gu], BF(0.0))
        oh = np.zeros((NCLS, NCOL), np.float32)
        oh[tu[valid], np.nonzero(valid)[0]] = 1.0
        pk_np[2 : 2 + NCLS, AUG : AUG + NCOL] = (-C * oh).astype(BF)
        # psbp blocks: per tile, 2C*mask with diag overwritten to DIAG
        for mt in range(MT):
            tr = tu[SPL + mt * P : SPL + mt * P + P]  # row classes
            tc_ = tu[mt * P : mt * P + W]  # window col classes
            m = (tr[:, None] == tc_[None, :]) & (tr[:, None] >= 0)
            blk = np.where(m, np.float32(2.0 * C), np.float32(0.0))
            blk[np.arange(P), SPL + np.arange(P)] = np.float32(DIAG)
            pk_np[:, PSBP + mt * W : PSBP + (mt + 1) * W] = blk.astype(BF)
        pk_np[np.arange(P), EYE + np.arange(P)] = BF(1.0)

        in_maps.append({"xt": xt_np, "pk": pk_np})
        meta.append(rows)
    return in_maps, meta, sq, (SPL, W, NCOL)


def _assemble(results, meta, sq):
    far2 = np.empty(N, np.float64)
    near2 = np.empty(N, np.float64)
    for c0 in range(NCORES):
        r = np.asarray(results[c0]["res"], np.float64)  # [P, 2*MT]
        rows = meta[c0]
        for mt in range(MT):
            g = rows[mt * P : (mt + 1) * P]
            far2[g] = sq[g] - C - r[:, mt]
            near2[g] = sq[g] + C - r[:, MT + mt]
    far = np.sqrt(np.maximum(far2, 0.0))
    near = np.sqrt(np.maximum(near2, 0.0))
    loss = np.float32(np.mean(np.maximum(far - near, 0.0)))
    return np.asarray(loss, np.float32)


def run_kernel(inputs, targets, trace=False):
    """Returns (loss, BassKernelResults)."""
    from concourse.bass_utils import run_bass_kernel_spmd

    global _compiled
    in_maps, meta, sq, key = _prep(inputs, targets)
    if _compiled is None or _compiled[0] != key:
        _compiled = (key, _build_nc(*key))
    nc = _compiled[1]
    br = run_bass_kernel_spmd(
        nc, in_maps, core_ids=list(range(NCORES)), trace=trace
    )
    return _assemble(br.results, meta, sq), br


def kernel(inputs, targets):
    loss, _ = run_kernel(inputs, targets)
    return loss


# revision 16
# speedup vs baseline: 1.2639x; 1.2412x over previous
"""Trainium2 Bass kernel for a pairwise-distance cluster margin loss.

Math (matches the jax reference):
    far_i  = max_{j: t_j=t_i} dist_ij
    near_i = second smallest dist_ij over class(i)  (smallest is self)
    loss   = mean(relu(far - near))

Key insight: the loss only involves SAME-CLASS distances.  With rows
sorted by class, each 128-row tile's class-mates lie within a narrow
band of the sorted order (max class size ~82), so each tile only needs
W ~ 264 columns instead of 4096 -> ~14x less GEMM work than the full
distance matrix.

Per core (512 sorted rows): the column "universe" is the sorted slice
order[512c-SPL : 512c-SPL+NCOL] (padded with zeros at the array ends).
Row-tile mt multiplies against universe cols [128mt, 128mt+W).  A single
fp8 tensor xt8 = fp8(sqrt2*x[universe])^T serves as BOTH matmul operands
(lhsT slice = own rows, rhs slice = window), so the PE computes
    psA = 2 x_i.x_j - sq_j - C*mask      (fp8 DR chain + one bf16 aug)
and the stats flip max<->min versus the usual formulation:
    rowmin(psA)                   -> far2  = sq_i - C - fstat
    rowmax(psA + 2C*mask + Ddiag) -> near2 = sq_i + C - gstat
The mask/diag term is a host-precomputed bf16 SBUF tensor added on the
DVE (no second matmul chain, no scalar-engine copy).  Host applies
sqrt / relu / mean on the 4096 reduced stats.
"""

import numpy as np
import ml_dtypes

BF = ml_dtypes.bfloat16
F8 = ml_dtypes.float8_e4m3

N = 4096  # rows (points)
D = 2048  # feature dim
P = 128  # partitions
NCORES = 8
MB = N // NCORES  # 512 rows per core
KX = D // P  # 16 x-chunks of 128
MT = MB // P  # 4 row tiles of 128 per core
NCLS = 64

C = float(2.0**17)  # mask offset; > max |2xixj - sqj| (~15k)
DIAG = -float(2.0**31)  # diagonal push-out

_compiled = None  # (key, nc)


def _spill(ts):
    """Max class-band spill (left, right) over all 128-row windows of the
    class-sorted target vector ts."""
    spl = spr = 0
    nw = N // P
    for w in range(nw):
        lo_cls = ts[w * P]
        hi_cls = ts[w * P + P - 1]
        lo = int(np.searchsorted(ts, lo_cls, "left"))
        hi = int(np.searchsorted(ts, hi_cls, "right"))
        spl = max(spl, w * P - lo)
        spr = max(spr, hi - (w * P + P))
    return spl, spr


def _build_nc(SPL, W, NCOL):
    import concourse.mybir as mybir
    import concourse.tile as tile
    from concourse import bacc

    nc = bacc.Bacc("TRN2", target_bir_lowering=False)
    f32 = mybir.dt.float32
    bf16 = mybir.dt.bfloat16
    fp8 = mybir.dt.float8e4
    DR = mybir.MatmulPerfMode.DoubleRow
    X = mybir.AxisListType.X
    MIN = mybir.AluOpType.min

    # packed bf16 tensor: [lhsa (MB) | aug (NCOL) | eye (P) | psbp (MT*W)]
    LHSA = 0
    AUG = MB
    EYE = MB + NCOL
    PSBP = MB + NCOL + P
    PK = MB + NCOL + P + MT * W

    xt_d = nc.dram_tensor("xt", [P, KX, NCOL], fp8, kind="ExternalInput")
    pk_d = nc.dram_tensor("pk", [P, PK], bf16, kind="ExternalInput")
    res_d = nc.dram_tensor("res", [P, 2 * MT], f32, kind="ExternalOutput")

    with tile.TileContext(nc) as tc:
        with (
            tc.tile_pool(name="singles", bufs=1) as singles,
            tc.tile_pool(name="psa", bufs=1, space="PSUM") as psa,
        ):
            xt = singles.tile([P, KX, NCOL], fp8)
            pk = singles.tile([P, PK], bf16)
            # DMA bandwidth is shared across queues (~300 B/ns aggregate),
            # so prioritize xt in chunk order (PE consumes chunk-major);
            # the pk blocks are only needed at the end, psbp last of all
            nc.sync.dma_start(out=xt[:, 0:2, :], in_=xt_d[:, 0:2, :])
            nc.scalar.dma_start(out=xt[:, 2:4, :], in_=xt_d[:, 2:4, :])
            nc.sync.dma_start(out=xt[:, 4:8, :], in_=xt_d[:, 4:8, :])
            nc.scalar.dma_start(out=xt[:, 8:12, :], in_=xt_d[:, 8:12, :])
            nc.sync.dma_start(out=xt[:, 12:KX, :], in_=xt_d[:, 12:KX, :])
            nc.scalar.dma_start(out=pk[:, 0:PSBP], in_=pk_d[:, 0:PSBP])
            nc.sync.dma_start(out=pk[:, PSBP:PK], in_=pk_d[:, PSBP:PK])

            fg = singles.tile([P, 2 * MT], f32, name="fg")

            at = [psa.tile([P, 512], f32, name=f"a{mt}") for mt in range(MT)]

            # chunk-major fp8 DR chains: the PE consumes chunk pairs in
            # DMA arrival order across all four PSUM banks (no mid stalls)
            for c in range(0, KX, 2):
                for mt in range(MT):
                    nc.tensor.matmul(
                        at[mt][:, 0:W],
                        xt[:, c : c + 2, SPL + mt * P : SPL + mt * P + P],
                        xt[:, c : c + 2, mt * P : mt * P + W],
                        start=(c == 0),
                        stop=False,
                        perf_mode=DR,
                    )

            def aug(mt):
                nc.tensor.matmul(
                    at[mt][:, 0:W],
                    pk[:, LHSA + mt * P : LHSA + (mt + 1) * P],
                    pk[:, AUG + mt * P : AUG + mt * P + W],
                    start=False,
                    stop=True,
                )

            def redmin(mt):
                # far stat: rowmin of psA (diag never wins the min)
                nc.vector.tensor_reduce(
                    fg[:, mt : mt + 1], at[mt][:, 0:W], axis=X, op=MIN
                )

            def eyeadd(mt):
                # after the f-reduce has read psA, accumulate the
                # 2C*mask + DIAG*diag term into the same PSUM bank on the
                # PE (WAR dep is tracked by the tile framework)
                nc.tensor.matmul(
                    at[mt][:, 0:W],
                    pk[:, EYE : EYE + P],
                    pk[:, PSBP + mt * W : PSBP + (mt + 1) * W],
                    start=False,
                    stop=True,
                )

            def redmax(mt):
                # near stat: plain rowmax of psA after the mask/diag add
                nc.vector.reduce_max(
                    fg[:, MT + mt : MT + mt + 1], at[mt][:, 0:W], axis=X
                )

            aug(0)
            aug(1)
            redmin(0)
            eyeadd(0)
            aug(2)
            redmin(1)
            eyeadd(1)
            aug(3)
            redmin(2)
            redmax(0)
            eyeadd(2)
            redmin(3)
            redmax(1)
            eyeadd(3)
            redmax(2)
            redmax(3)

            nc.sync.dma_start(out=res_d[:, :], in_=fg)

    nc.compile()
    return nc


def _prep(x, t):
    x = np.asarray(x, np.float32)
    t = np.asarray(t).astype(np.int64)
    order = np.argsort(t, kind="stable")
    ts = t[order]
    spl, spr = _spill(ts)
    W = ((P + spl + spr) + 7) // 8 * 8
    NCOL = (MB + (W - P) + 31) // 32 * 32  # chunk-pair offsets stay 64B-aligned
    SPL = spl

    q8 = (np.float32(np.sqrt(2.0)) * x).astype(F8)  # [N, D]
    sq = np.sum(x.astype(np.float64) ** 2, axis=1)
    sqhi = sq.astype(BF)
    sqlo = (sq - sqhi.astype(np.float64)).astype(BF)

    LHSA = 0
    AUG = MB
    EYE = MB + NCOL
    PSBP = MB + NCOL + P
    PK = MB + NCOL + P + MT * W

    in_maps = []
    meta = []
    for c0 in range(NCORES):
        u0 = c0 * MB - SPL
        uidx = np.arange(u0, u0 + NCOL)
        valid = (uidx >= 0) & (uidx < N)
        gu = order[np.clip(uidx, 0, N - 1)]
        tu = np.where(valid, ts[np.clip(uidx, 0, N - 1)], -1)

        xt_cols = q8[gu].T.copy()  # [D, NCOL]
        xt_cols[:, ~valid] = F8(0.0)
        xt_np = np.ascontiguousarray(
            xt_cols.reshape(KX, P, NCOL).transpose(1, 0, 2)
        )

        pk_np = np.zeros((P, PK), BF)
        # lhsa block: row0 = row1 = 1, rows 2+c = onehot(t_row)
        rows = order[c0 * MB : (c0 + 1) * MB]
        ohr = np.zeros((NCLS, MB), np.float32)
        ohr[t[rows], np.arange(MB)] = 1.0
        pk_np[0, LHSA : LHSA + MB] = BF(1.0)
        pk_np[1, LHSA : LHSA + MB] = BF(1.0)
        pk_np[2 : 2 + NCLS, LHSA : LHSA + MB] = ohr.astype(BF)
        # aug block: row0 = -sqhi_j, row1 = -sqlo_j, rows 2+c = -C*onehot
        pk_np[0, AUG : AUG + NCOL] = np.where(valid, -sqhi[gu], BF(0.0))
        pk_np[1, AUG : AUG + NCOL] = np.where(valid, -sqlo[gu], BF(0.0))
        oh = np.zeros((NCLS, NCOL), np.float32)
        oh[tu[valid], np.nonzero(valid)[0]] = 1.0
        pk_np[2 : 2 + NCLS, AUG : AUG + NCOL] = (-C * oh).astype(BF)
        # psbp blocks: per tile, 2C*mask with diag overwritten to DIAG
        for mt in range(MT):
            tr = tu[SPL + mt * P : SPL + mt * P + P]  # row classes
            tc_ = tu[mt * P : mt * P + W]  # window col classes
            m = (tr[:, None] == tc_[None, :]) & (tr[:, None] >= 0)
            blk = np.where(m, np.float32(2.0 * C), np.float32(0.0))
            blk[np.arange(P), SPL + np.arange(P)] = np.float32(DIAG)
            pk_np[:, PSBP + mt * W : PSBP + (mt + 1) * W] = blk.astype(BF)
        pk_np[np.arange(P), EYE + np.arange(P)] = BF(1.0)

        in_maps.append({"xt": xt_np, "pk": pk_np})
        meta.append(rows)
    return in_maps, meta, sq, (SPL, W, NCOL)


def _assemble(results, meta, sq):
    far2 = np.empty(N, np.float64)
    near2 = np.empty(N, np.float64)
    for c0 in range(NCORES):
        r = np.asarray(results[c0]["res"], np.float64)  # [P, 2*MT]
        rows = meta[c0]
        for mt in range(MT):
            g = rows[mt * P : (mt + 1) * P]
            far2[g] = sq[g] - C - r[:, mt]
            near2[g] = sq[g] + C - r[:, MT + mt]
    far = np.sqrt(np.maximum(far2, 0.0))
    near = np.sqrt(np.maximum(near2, 0.0))
    loss = np.float32(np.mean(np.maximum(far - near, 0.0)))
    return np.asarray(loss, np.float32)


def run_kernel(inputs, targets, trace=False):
    """Returns (loss, BassKernelResults)."""
    from concourse.bass_utils import run_bass_kernel_spmd

    global _compiled
    in_maps, meta, sq, key = _prep(inputs, targets)
    if _compiled is None or _compiled[0] != key:
        _compiled = (key, _build_nc(*key))
    nc = _compiled[1]
    br = run_bass_kernel_spmd(
        nc, in_maps, core_ids=list(range(NCORES)), trace=trace
    )
    return _assemble(br.results, meta, sq), br


def kernel(inputs, targets):
    loss, _ = run_kernel(inputs, targets)
    return loss
